# revision 19
# baseline (speedup 1.0000x reference)
"""Trainium2 Bass kernel for nn_CausalNet (block-diagonal GNN + BN + MLP head).

Data-parallel over batch (8 samples/core on 8 cores). v3 design:
 - bf16 matmuls everywhere in the GNN body (fp32 PSUM accumulation).
 - No gpsimd partition reduces: row norms and degrees via ones-vector
   matmuls, rsqrt via Sqrt + reciprocal_approx_fast.
 - Loop-split schedule keeps the PE queue unblocked: all Gram matmuls,
   then rank-1 norm outers, then all XW1 (interleaved with the vector
   adjacency chain), then degrees.
 - BN sums and sum-of-squares both fused into the PSUM evacuation path
   (per-group accum_out), so only 2 tiny reduces sit before each
   AllReduce.
 - Wm1 (bf16, 4.2MB/core) SBUF-resident via one early DMA; readout
   matmuls consume SBUF directly.
 - AllToAll split into two sample-halves so the first half exchanges
   while the second half of layer 2 still computes; BN2+relu applied
   post-exchange on the received tiles.
"""
import sys
import numpy as np

sys.path.insert(0, "/opt/trn_rl_repo")

B, N, P, D = 64, 4, 128, 256
H = 256
TOTP = N * P          # 512
NCORES = 8
BLOC = B // NCORES    # 8 samples per core
T = BLOC * TOTP       # 4096 tokens per core
NB = BLOC * N         # 32 (sample, subgraph) blocks per core
SB = NB // 4          # 8 groups of 4 blocks (group g == sample g)
FEAT = TOTP * H       # 131072
FSL = FEAT // NCORES  # 16384 Wm1 rows per core
TSL = TOTP // NCORES  # 64 patches per a2a slice
JT = H // 128         # 2 feature partition-tiles
NTAU = JT * TSL       # 128 readout k-tiles
HB = BLOC // 2        # 4 samples per a2a half
EPS_BN = 1e-5
CNT1 = float(B * TOTP)   # BN denominator for GCN layers
CNT2 = float(B)          # BN denominator for head


def build_bass(no_cc=False):
    import concourse.bass as bass
    import concourse.bacc as bacc
    import concourse.mybir as mybir
    import concourse.tile as tile

    f32 = mybir.dt.float32
    bf16 = mybir.dt.bfloat16
    u16 = mybir.dt.uint16
    Act = mybir.ActivationFunctionType
    Alu = mybir.AluOpType
    AX = mybir.AxisListType

    nc = bacc.Bacc("TRN2", target_bir_lowering=False, debug=False,
                   num_devices=NCORES)

    def inp(name, shape, dt=f32):
        return nc.dram_tensor(name, shape, dt, kind="ExternalInput")

    xT_d = inp("xT", [D, T], bf16)       # d-major activations, bf16
    W1_d = inp("W1", [D, H], bf16)
    b1_d = inp("b1r", [1, H], bf16)
    W2_d = inp("W2", [H, H], bf16)
    b2_d = inp("b2r", [1, H], bf16)
    g1_d = inp("g1p", [128, JT])
    be1_d = inp("be1p", [128, JT])
    g2_d = inp("g2p", [128, JT])
    be2_d = inp("be2p", [128, JT])
    mAT_d = inp("mAT", [P, 4 * P])       # (0.5*mask*(1-I)).T tiled x4
    mBT_d = inp("mBT", [P, 4 * P])       # mAT + I tiled x4
    Wm1_d = inp("Wm1s", [128, NTAU * 128], bf16)  # [f_lane, (jh,pt,o)]
    gm1_d = inp("gm1", [128, 1])
    bem1_d = inp("bem1", [128, 1])
    Wm2_d = inp("Wm2", [128, 64])
    gm2_d = inp("gm2", [64, 1])
    bem2_d = inp("bem2", [64, 1])
    Wm3_d = inp("Wm3", [64, 2])
    bm3_d = inp("bm3", [2, 1])
    onerb_d = inp("ones_row16", [1, 128], bf16)
    onecb_d = inp("ones_col16", [128, 1], bf16)
    id_d = inp("ident", [128, 128])

    out_ext = nc.dram_tensor("out", [2, B], f32, kind="ExternalOutput")

    with tile.TileContext(nc) as tc:
        with (
            tc.tile_pool(name="persist", bufs=1) as pp,
            tc.tile_pool(name="work", bufs=3) as wp,
            tc.tile_pool(name="sqc", bufs=3) as sqcp,
            tc.tile_pool(name="xw", bufs=34) as xwp,
            tc.tile_pool(name="small", bufs=2) as sp,
            tc.tile_pool(name="rows", bufs=1) as rp,
            tc.tile_pool(name="ps", bufs=2, space="PSUM") as ps,
            tc.tile_pool(name="dram", bufs=1, space="DRAM") as dp,
        ):
            # ---------------- persistent SBUF / initial DMAs ----------------
            def load(name, dram, shape, dt=f32, sl=None, eng=None):
                t = pp.tile(shape, dt, tag=name, name=name)
                e = eng or nc.sync
                e.dma_start(out=t[:], in_=dram[:] if sl is None else sl)
                return t

            # Wm1 first: 4.2MB, overlaps the whole GCN phase (gpsimd queue).
            Wm1s = load("Wm1s", Wm1_d, [128, NTAU * 128], bf16, eng=nc.gpsimd)
            xbf = [load(f"xT{k}", xT_d, [128, T], bf16,
                        xT_d[k * 128:(k + 1) * 128, :])
                   for k in range(2)]
            W1s = [load(f"W1{k}", W1_d, [128, H], bf16,
                        W1_d[k * 128:(k + 1) * 128, :], eng=nc.scalar)
                   for k in range(2)]
            W2s = [load(f"W2{k}", W2_d, [128, H], bf16,
                        W2_d[k * 128:(k + 1) * 128, :], eng=nc.scalar)
                   for k in range(2)]
            b1s = load("b1", b1_d, [1, H], bf16, eng=nc.scalar)
            b2s = load("b2", b2_d, [1, H], bf16, eng=nc.scalar)
            mATs = load("mAT", mAT_d, [P, 4 * P], eng=nc.sync)
            mBTs = load("mBT", mBT_d, [P, 4 * P], eng=nc.sync)
            onerb = load("onerb", onerb_d, [1, 128], bf16, eng=nc.scalar)
            onecb = load("onecb", onecb_d, [128, 1], bf16, eng=nc.scalar)
            idents = load("ident", id_d, [128, 128], eng=nc.sync)
            g1s = load("g1", g1_d, [128, JT], eng=nc.sync)
            be1s = load("be1", be1_d, [128, JT], eng=nc.sync)
            g2s = load("g2", g2_d, [128, JT], eng=nc.sync)
            be2s = load("be2", be2_d, [128, JT], eng=nc.sync)
            gm1s = load("gm1", gm1_d, [128, 1], eng=nc.sync)
            bem1s = load("bem1", bem1_d, [128, 1], eng=nc.sync)
            gm2s = load("gm2", gm2_d, [64, 1], eng=nc.sync)
            bem2s = load("bem2", bem2_d, [64, 1], eng=nc.sync)
            Wm2s = load("Wm2", Wm2_d, [128, 64], eng=nc.sync)
            Wm3s = load("Wm3", Wm3_d, [64, 2], eng=nc.sync)
            bm3s = load("bm3", bm3_d, [2, 1], eng=nc.sync)

            epsb = pp.tile([128, 1], f32, tag="epsb")
            nc.vector.memset(epsb[:], EPS_BN)

            rinv = pp.tile([1, T], bf16, tag="rinv")     # 1/||x_t|| rows
            dinv = pp.tile([1, T], bf16, tag="dinv")     # deg^-1/2 rows
            AT_all = pp.tile([128, T], bf16, tag="ATall")  # A^T blocks
            AnT = pp.tile([128, T], bf16, tag="AnT")       # normalized A^T
            h1T = pp.tile([128, 2 * T], bf16, tag="h1T")   # [f, (jh, t)]
            h2T = pp.tile([128, 2 * T], bf16, tag="h2T")
            recv = [pp.tile([128, T], bf16, tag=f"recv{j}", name=f"recv{j}")
                    for j in range(JT)]
            sumb1 = pp.tile([128, 16], f32, tag="sumb1")   # per-group sums
            sumb2 = pp.tile([128, 16], f32, tag="sumb2")
            sqb1 = pp.tile([128, 16], f32, tag="sqb1")     # per-group sumsq
            sqb2 = pp.tile([128, 16], f32, tag="sqb2")

            rg = [list(range(NCORES))]
            qs = [nc.sync, nc.scalar, nc.gpsimd]

            def cc(kind, op, cin, cout):
                if no_cc:
                    nc.sync.dma_start(out=cout[:], in_=cin[:])
                else:
                    nc.gpsimd.collective_compute(
                        kind, op, replica_groups=rg,
                        ins=[cin.opt()], outs=[cout.opt()])

            st1_in = dp.tile([128, 4], f32, tag="st1i", name="st1_in")
            st1_out = dp.tile([NCORES, 128, 4], f32, tag="st1o",
                              addr_space="Shared", name="st1_out")
            st2_in = dp.tile([128, 4], f32, tag="st2i", name="st2_in")
            st2_out = dp.tile([NCORES, 128, 4], f32, tag="st2o",
                              addr_space="Shared", name="st2_out")
            # a2a payload (two sample-halves): [slot, jh, f, s, pt] — per
            # (slot, jh, f) the (s, pt) run is 512B-contiguous.
            a2a_in = [dp.tile([NCORES, JT, 128, HB * TSL + 4 * h], bf16,
                              tag=f"a2ai{h}", name=f"a2a_in{h}")
                      for h in range(2)]
            a2a_out = [dp.tile([NCORES, JT, 128, HB * TSL + 4 * h], bf16,
                               tag=f"a2ao{h}", name=f"a2a_out{h}")
                       for h in range(2)]
            z1_in = dp.tile([128, 64], f32, tag="z1i", name="z1_in")
            z1_out = dp.tile([NCORES, 128, 64], f32, tag="z1o",
                             addr_space="Shared", name="z1_out")

            # -------- row norms: rinv[t] = 1/||x_t||  (chunked) -------------
            rn_sb = rp.tile([1, T], f32, tag="rowf")
            for j in range(SB):
                ns = ps.tile([1, 512], f32, tag="rowmm")
                for kt in range(2):
                    sqx = sqcp.tile([128, 512], bf16, tag="sqx")
                    nc.scalar.activation(
                        sqx[:], xbf[kt][:, j * 512:(j + 1) * 512], Act.Square)
                    nc.tensor.matmul(ns[:], onecb[:], sqx[:],
                                     start=(kt == 0), stop=(kt == 1))
                sl = slice(j * 512, (j + 1) * 512)
                nc.scalar.activation(rn_sb[:, sl], ns[:], Act.Sqrt)
                nc.vector.reciprocal_approx_fast(rn_sb[:, sl], rn_sb[:, sl])
                nc.scalar.activation(rinv[:, sl], rn_sb[:, sl], Act.Copy)

            # ---- Gram + norm outers per group; adjacency chain trails ------
            for g in range(SB):
                c0 = g * 4 * P
                G4 = ps.tile([P, 4 * P], f32, tag="big", bufs=4)
                for b in range(4):
                    cb = c0 + b * P
                    for kt in range(2):
                        nc.tensor.matmul(
                            G4[:, b * P:(b + 1) * P],
                            xbf[kt][:, cb:cb + P], xbf[kt][:, cb:cb + P],
                            start=(kt == 0), stop=(kt == 1))
                R4 = ps.tile([P, 4 * P], f32, tag="big", bufs=4)
                for b in range(4):
                    cb = c0 + b * P
                    nc.tensor.matmul(R4[:, b * P:(b + 1) * P],
                                     rinv[:, cb:cb + P], rinv[:, cb:cb + P],
                                     start=True, stop=True)
                tt = wp.tile([P, 4 * P], f32, tag="tt")
                nc.vector.tensor_mul(tt[:], G4[:], mATs[:])
                nc.vector.tensor_mul(tt[:], R4[:], tt[:])
                # gpsimd add: SBUF-only operands, keeps the vector queue free
                nc.gpsimd.tensor_add(AT_all[:, c0:c0 + 4 * P], tt[:], mBTs[:])

            # -------- XW1 for all blocks (overlaps the vector chain) --------
            def xw_block(cb, src_fn, Ws, bs):
                """[128 q, 256 f] = x_block.T @ W + 1 b   (bf16)."""
                xw_ps = ps.tile([128, H], f32, tag="xwps", bufs=2)
                for kt in range(2):
                    nc.tensor.matmul(
                        xw_ps[:], src_fn(kt, cb),
                        Ws[kt][:], start=(kt == 0), stop=False)
                nc.tensor.matmul(xw_ps[:], onerb[:], bs[:],
                                 start=False, stop=True)
                xw = xwp.tile([128, H], bf16, tag="xw")
                nc.vector.tensor_copy(xw[:], xw_ps[:])
                return xw

            xws1 = [xw_block(bb * P, lambda kt, cb: xbf[kt][:, cb:cb + P],
                             W1s, b1s)
                    for bb in range(NB)]

            # -------- degrees + dinv (chunk-wise) ---------------------------
            dg_sb = rp.tile([1, T], f32, tag="rowf")
            for g in range(SB):
                c0 = g * 4 * P
                dg = ps.tile([1, 512], f32, tag="rowmm")
                nc.tensor.matmul(dg[:], onecb[:], AT_all[:, c0:c0 + 4 * P],
                                 start=True, stop=True)
                sl = slice(c0, c0 + 4 * P)
                nc.scalar.activation(dg_sb[:, sl], dg[:], Act.Sqrt)
                nc.vector.reciprocal_approx_fast(dg_sb[:, sl], dg_sb[:, sl])
                nc.scalar.activation(dinv[:, sl], dg_sb[:, sl], Act.Copy)

            # -------- An^T + bmm layer core ---------------------------------
            def hh_group(g, xws, hT, sumb, sqb):
                c0 = g * 4 * P
                for jh in range(JT):
                    hh4 = ps.tile([128, 4 * P], f32, tag="big", bufs=4)
                    for b in range(4):
                        cb = c0 + b * P
                        nc.tensor.matmul(
                            hh4[:, b * P:(b + 1) * P],
                            xws[g * 4 + b][:, jh * 128:(jh + 1) * 128],
                            AnT[:, cb:cb + P],
                            start=True, stop=True)
                    col = g * 2 + jh
                    nc.scalar.activation(
                        hT[:, jh * T + c0: jh * T + c0 + 4 * P], hh4[:],
                        Act.Identity, accum_out=sumb[:, col:col + 1])
                    sqh = sqcp.tile([128, 512], bf16, tag="sqh")
                    nc.scalar.activation(sqh[:], hh4[:], Act.Square,
                                         accum_out=sqb[:, col:col + 1])

            # layer 1: Do4 + AnT per group, then hh
            for g in range(SB):
                c0 = g * 4 * P
                Do4 = ps.tile([P, 4 * P], f32, tag="big", bufs=4)
                for b in range(4):
                    cb = c0 + b * P
                    nc.tensor.matmul(Do4[:, b * P:(b + 1) * P],
                                     dinv[:, cb:cb + P], dinv[:, cb:cb + P],
                                     start=True, stop=True)
                nc.vector.tensor_mul(AnT[:, c0:c0 + 4 * P],
                                     AT_all[:, c0:c0 + 4 * P], Do4[:])
                hh_group(g, xws1, h1T, sumb1, sqb1)

            # ---------------- BN stats + allreduce + apply ------------------
            def stat_reduce(sumb, sqb):
                st = sp.tile([128, 4], f32, tag="st")
                for jh in range(JT):
                    nc.vector.reduce_sum(
                        st[:, 2 * jh:2 * jh + 1],
                        sumb[:].rearrange("f (g j) -> f j g", j=2)[:, jh, :],
                        AX.X)
                    nc.vector.reduce_sum(
                        st[:, 2 * jh + 1:2 * jh + 2],
                        sqb[:].rearrange("f (g j) -> f j g", j=2)[:, jh, :],
                        AX.X)
                return st

            def bn_stats(sumb, sqb, stin, stout):
                st = stat_reduce(sumb, sqb)
                nc.sync.dma_start(out=stin[:], in_=st[:])
                cc("AllGather", Alu.bypass, stin, stout)
                stga = sp.tile([128, 32], f32, tag="stga")
                for src_c in range(NCORES):
                    qs[src_c % 3].dma_start(
                        out=stga[:, src_c * 4:(src_c + 1) * 4],
                        in_=stout[src_c])
                stg = sp.tile([128, 4], f32, tag="stg")
                nc.vector.reduce_sum(
                    stg[:],
                    stga[:].rearrange("f (s c) -> f c s", s=NCORES), AX.X)
                return stg

            def bn_coeffs(stg, jh, gs, bes):
                mean = sp.tile([128, 1], f32, tag="mean")
                nc.vector.tensor_scalar_mul(mean[:], stg[:, 2 * jh:2 * jh + 1],
                                            1.0 / CNT1)
                msq = sp.tile([128, 1], f32, tag="msq")
                nc.vector.tensor_mul(msq[:], mean[:], mean[:])
                var = sp.tile([128, 1], f32, tag="var")
                nc.vector.tensor_scalar_mul(var[:],
                                            stg[:, 2 * jh + 1:2 * jh + 2],
                                            1.0 / CNT1)
                nc.vector.tensor_sub(var[:], var[:], msq[:])
                sd = sp.tile([128, 1], f32, tag="sd")
                nc.scalar.activation(sd[:], var[:], Act.Sqrt, bias=epsb[:])
                rsd = sp.tile([128, 1], f32, tag="rsd")
                nc.vector.reciprocal(rsd[:], sd[:])
                a = sp.tile([128, 1], f32, tag="a")
                nc.vector.tensor_mul(a[:], gs[:, jh:jh + 1], rsd[:])
                c = sp.tile([128, 1], f32, tag="c")
                nc.vector.tensor_mul(c[:], mean[:], a[:])
                nc.vector.tensor_sub(c[:], bes[:, jh:jh + 1], c[:])
                return a, c

            stg1 = bn_stats(sumb1, sqb1, st1_in, st1_out)
            for jh in range(JT):
                a, c = bn_coeffs(stg1, jh, g1s, be1s)
                nc.scalar.activation(h1T[:, jh * T:(jh + 1) * T],
                                     h1T[:, jh * T:(jh + 1) * T],
                                     Act.Relu, bias=c[:], scale=a[:])

            # ------- layer 2, halved, with A2A pipelined ---------------------
            hv = h2T[:].rearrange("f (jh s dd p) -> f jh s dd p",
                                  jh=JT, s=BLOC, dd=NCORES, p=TSL)

            def stage_half(h, st2loc=None):
                s0 = h * HB
                for dd in range(NCORES):
                    for jh in range(JT):
                        qs[(dd * JT + jh) % 3].dma_start(
                            out=a2a_in[h][dd, jh, :, 0:HB * TSL]
                            .rearrange("f (s p) -> f s p", s=HB),
                            in_=hv[:, jh, s0:s0 + HB, dd, :])
                        if st2loc is not None:
                            qs[dd % 3].dma_start(
                                out=a2a_in[h][dd, jh, :,
                                              HB * TSL:HB * TSL + 4]
                                .bitcast(u16),
                                in_=st2loc[:, 2 * jh:2 * jh + 2]
                                .bitcast(u16))
                cc("AllToAll", Alu.bypass, a2a_in[h], a2a_out[h])

            src2 = lambda kt, cb: h1T[:, kt * T + cb: kt * T + cb + P]
            st2loc = None
            for h in range(2):
                for g in range(h * 4, h * 4 + 4):
                    xws2 = {g * 4 + b: xw_block((g * 4 + b) * P, src2,
                                                W2s, b2s)
                            for b in range(4)}
                    hh_group(g, xws2, h2T, sumb2, sqb2)
                if h == 1:
                    st2loc = stat_reduce(sumb2, sqb2)
                stage_half(h, st2loc if h == 1 else None)

            for h in range(2):
                for src in range(NCORES):
                    for jh in range(JT):
                        qs[(src * JT + jh) % 3].dma_start(
                            out=recv[jh][:, src * 512 + h * 256:
                                         src * 512 + h * 256 + 256]
                            .rearrange("f (s p) -> f s p", s=HB),
                            in_=a2a_out[h][src, jh, :, 0:HB * TSL]
                            .rearrange("f (s p) -> f s p", s=HB))
            stga2 = sp.tile([128, 64], bf16, tag="stga2")
            for src_c in range(NCORES):
                for jh in range(JT):
                    qs[(src_c + jh) % 3].dma_start(
                        out=stga2[:, src_c * 8 + jh * 4:
                                  src_c * 8 + jh * 4 + 4].bitcast(u16),
                        in_=a2a_out[1][src_c, jh, :,
                                       HB * TSL:HB * TSL + 4].bitcast(u16))
            st2g = sp.tile([128, 4], f32, tag="st2g")
            nc.vector.reduce_sum(
                st2g[:],
                stga2[:].bitcast(f32).rearrange("f (s q) -> f q s",
                                                s=NCORES), AX.X)

            # BN2 + relu applied on the received (redistributed) tiles.
            for jh in range(JT):
                a, c = bn_coeffs(st2g, jh, g2s, be2s)
                nc.scalar.activation(recv[jh][:], recv[jh][:],
                                     Act.Relu, bias=c[:], scale=a[:])

            # ---------------- readout: z1 partial [64, 128] -----------------
            z1p_t = ps.tile([128, 512], f32, tag="big", bufs=4, name="z1p_t")
            z1p = z1p_t[:64, :128]
            rvs = [recv[jh][:].rearrange("f (src hh s p) -> f src hh s p",
                                         src=NCORES, hh=2, s=HB, p=TSL)
                   for jh in range(JT)]
            for jh in range(JT):
                for pt in range(TSL):
                    tau = jh * TSL + pt
                    nc.tensor.matmul(
                        z1p, rvs[jh][:, :, :, :, pt],
                        Wm1s[:, tau * 128:(tau + 1) * 128],
                        start=(tau == 0), stop=(tau == NTAU - 1))
            z1s = sp.tile([64, 128], f32, tag="z1s")
            nc.vector.tensor_copy(z1s[:], z1p)

            # transpose the partial BEFORE the allreduce
            z1tp_t = ps.tile([128, 512], f32, tag="big", bufs=4, name="z1tp_t")
            z1tp = z1tp_t[:, :64]
            nc.tensor.transpose(z1tp, z1s[:], idents[:64, :64])
            z1ts = sp.tile([128, 64], f32, tag="z1ts")
            nc.vector.tensor_copy(z1ts[:], z1tp)
            nc.sync.dma_start(out=z1_in[:], in_=z1ts[:])
            cc("AllGather", Alu.bypass, z1_in, z1_out)
            z1cat = sp.tile([128, 512], f32, tag="z1cat")
            for src_c in range(NCORES):
                qs[src_c % 3].dma_start(
                    out=z1cat[:, src_c * 64:(src_c + 1) * 64],
                    in_=z1_out[src_c])
            z1t = sp.tile([128, 64], f32, tag="z1t")
            nc.vector.reduce_sum(
                z1t[:],
                z1cat[:].rearrange("o (s b) -> o b s", s=NCORES), AX.X)

            # ---------------- head BN + relu ----------------
            def head_bn(zt, parts, gs, bes):
                stm = sp.tile([parts, 1], f32, tag="hstm")
                nc.vector.reduce_sum(stm[:], zt[:], AX.X)
                mean = sp.tile([parts, 1], f32, tag="hmean")
                nc.vector.tensor_scalar_mul(mean[:], stm[:], 1.0 / CNT2)
                sqs2 = sp.tile([parts, 64], f32, tag="hsq")
                sts = sp.tile([parts, 1], f32, tag="hsts")
                nc.scalar.activation(sqs2[:], zt[:], Act.Square,
                                     accum_out=sts[:])
                var = sp.tile([parts, 1], f32, tag="hvar")
                nc.vector.tensor_scalar_mul(var[:], sts[:], 1.0 / CNT2)
                msq = sp.tile([parts, 1], f32, tag="hmsq")
                nc.vector.tensor_mul(msq[:], mean[:], mean[:])
                nc.vector.tensor_sub(var[:], var[:], msq[:])
                sd = sp.tile([parts, 1], f32, tag="hsd")
                nc.scalar.activation(sd[:], var[:], Act.Sqrt,
                                     bias=epsb[:parts, :])
                rsd = sp.tile([parts, 1], f32, tag="hrsd")
                nc.vector.reciprocal(rsd[:], sd[:])
                a = sp.tile([parts, 1], f32, tag="ha")
                nc.vector.tensor_mul(a[:], gs[:], rsd[:])
                c = sp.tile([parts, 1], f32, tag="hc")
                nc.vector.tensor_mul(c[:], mean[:], a[:])
                nc.vector.tensor_sub(c[:], bes[:], c[:])
                nc.scalar.activation(zt[:], zt[:], Act.Relu, bias=c[:],
                                     scale=a[:])

            head_bn(z1t, 128, gm1s, bem1s)

            z2_t = ps.tile([128, 512], f32, tag="big", bufs=4, name="z2_t")
            z2_ps = z2_t[:64, :64]
            nc.tensor.matmul(z2_ps, Wm2s[:], z1t[:], start=True, stop=True)
            z2t = sp.tile([64, 64], f32, tag="z2t")
            nc.vector.tensor_copy(z2t[:], z2_ps)
            head_bn(z2t, 64, gm2s, bem2s)

            z3_t = ps.tile([128, 512], f32, tag="big", bufs=4, name="z3_t")
            z3_ps = z3_t[:2, :64]
            nc.tensor.matmul(z3_ps, Wm3s[:], z2t[:], start=True, stop=True)
            z3 = sp.tile([2, 64], f32, tag="z3")
            nc.vector.tensor_scalar_add(z3[:], z3_ps, bm3s[:])
            nc.sync.dma_start(out=out_ext[:], in_=z3[:])

    nc.finalize()
    return nc


_CACHE = {}


def prepare_in_maps(inputs):
    import ml_dtypes
    bf = ml_dtypes.bfloat16

    x = np.asarray(inputs["x"], np.float32)
    mask = np.asarray(inputs["edge_prior_mask"], np.float32)
    Wm1 = np.asarray(inputs["Wm1"], np.float32)

    mA = 0.5 * mask * (1.0 - np.eye(P, dtype=np.float32))
    mB = mA + np.eye(P, dtype=np.float32)

    def c2(v, parts):  # [2*parts] -> [parts, 2] column-per-tile packing
        return np.ascontiguousarray(
            np.asarray(v, np.float32).reshape(2, parts).T)

    common = {
        "W1": np.asarray(inputs["W1"], bf),
        "b1r": np.asarray(inputs["b1"], bf).reshape(1, H),
        "g1p": c2(inputs["g1"], 128), "be1p": c2(inputs["be1"], 128),
        "W2": np.asarray(inputs["W2"], bf),
        "b2r": np.asarray(inputs["b2"], bf).reshape(1, H),
        "g2p": c2(inputs["g2"], 128), "be2p": c2(inputs["be2"], 128),
        "mAT": np.ascontiguousarray(np.tile(mA.T, (1, 4))),
        "mBT": np.ascontiguousarray(np.tile(mB.T, (1, 4))),
        "gm1": np.asarray(inputs["gm1"], np.float32).reshape(128, 1),
        "bem1": np.asarray(inputs["bem1"], np.float32).reshape(128, 1),
        "Wm2": np.asarray(inputs["Wm2"], np.float32),
        "gm2": np.asarray(inputs["gm2"], np.float32).reshape(64, 1),
        "bem2": np.asarray(inputs["bem2"], np.float32).reshape(64, 1),
        "Wm3": np.asarray(inputs["Wm3"], np.float32),
        "bm3": np.asarray(inputs["bm3"], np.float32).reshape(2, 1),
        "ones_row16": np.ones((1, 128), bf),
        "ones_col16": np.ones((128, 1), bf),
        "ident": np.eye(128, dtype=np.float32),
    }
    in_maps = []
    for c in range(NCORES):
        xc = x[c * BLOC:(c + 1) * BLOC].reshape(T, D)
        m = dict(common)
        m["xT"] = np.ascontiguousarray(xc.T).astype(bf)
        # Wm1 rows for core c: (c*64+pt)*256 + jh*128 + f  ->  [f,(jh,pt,o)]
        Wc = Wm1[c * FSL:(c + 1) * FSL, :].reshape(TSL, JT, 128, 128)
        m["Wm1s"] = np.ascontiguousarray(
            Wc.transpose(2, 1, 0, 3).reshape(128, NTAU * 128)).astype(bf)
        in_maps.append(m)
    return in_maps


def kernel(**inputs):
    import concourse.bass_utils as bass_utils

    in_maps = prepare_in_maps(inputs)
    if "nc" not in _CACHE:
        _CACHE["nc"] = build_bass()
    res = bass_utils.run_bass_kernel_spmd(
        _CACHE["nc"], in_maps, core_ids=list(range(NCORES)))
    _CACHE["last"] = res
    out = res.results[0]["out"]  # [2, 64]
    return np.ascontiguousarray(np.asarray(out).T)
